# revision 55
# baseline (speedup 1.0000x reference)
"""Causal multi-head attention (B=2, S=2048, D=1024, H=16, Dh=64) on 8 trn2 cores.

Sharding: (batch, head-group) tensor parallel. Core c handles batch c//4 and
heads [4*(c%4), 4*(c%4)+4). Each core computes its 4 heads end-to-end
(QKV projections, causal softmax attention, W_O projection) and returns a
partial [S, D] output; the host sums the 4 partials per batch.

Per-core dataflow (v2 - cross-phase software pipeline):
  - Q^T, K^T produced in [Dh, S] layout so scores come out transposed
    (S^T[k, q]) and the softmax'd P~ needs no transpose for the P@V matmul.
  - Softmax denominator via a ones-column appended to V (M=65 matmuls):
    row 64 of the attention PSUM is the denominator.
  - Causal mask: multiplicative upper-triangular bf16 tile applied to the
    exp'd diagonal blocks on GpSimd.
  - ACT exp (1 elem/lane/cycle @1.2GHz, ~293ns/inst) is the intrinsic
    attention bottleneck and the per-chunk exp load grows with qc (causal
    trapezoid). v1 serialized "QKV(q); attention(q)" so the PE starved
    behind exp via PSUM-slot reuse, HAM re-throttled to 1.2GHz repeatedly.
  - v2 interleaves emission: attention(qc) kt-steps pop "fill" thunks
    between steps (QKV(qc+1) projection chains, V|ones groups, W_O(qc-2)
    output-projection pieces), so the PE always has slot-independent
    matmul work while ACT exps, and windows approach per-engine balance:
      window qc0: fills = V(0) groups + full QKV(1)
      window qc1: fills = QKV(2) + W_O(0)
      window qc2: fills = QKV(3) + W_O(1)   (QKV(3) PSUM copies on DVE)
      window qc3: fills = W_O(2)
  - W_O thunks hold ONE PSUM slot each (per 512-col half of out rows) so
    they never block the scores double-buffer.
  - PSUM->SBUF copy engine per window budget: exp owns ACT during heavy
    windows, so QKV(3) copies + softmax-denominator chain go to DVE;
    QKV(0..2) copies stay on ACT (spare there).
  - bf16 operands everywhere on the PE (fast weight load); fp32 PSUM
    accumulation; W_O stationary in float32r; softmax scale /8 inside the
    exp's free affine.
"""

import numpy as np

try:
    import concourse  # noqa: F401
except ImportError:  # pragma: no cover - harness containers stage it here
    import sys

    sys.path.insert(0, "/opt/trn_rl_repo")

B, S, D, H, DH = 2, 2048, 1024, 16, 64
NCORES = 8
HPC = 4  # heads per core
NPAIR = 2  # head pairs per core
SC = 512  # q-chunk width (scores matmul N)
NQC = S // SC  # 4 q-chunks
NST = S // 128  # 16 s/k/q tiles of 128
NDC = D // 128  # 8 contraction chunks of 128
VO_W = 65  # V columns + ones column
VO_QSTRIDE = 4 * VO_W  # per-head stride inside one quarter's V|ones tile

_cache = {}


def _build_program():
    from contextlib import ExitStack

    import concourse.mybir as mybir
    import concourse.tile as tile
    from concourse import bacc

    f32 = mybir.dt.float32
    f32r = mybir.dt.float32r
    bf16 = mybir.dt.bfloat16
    AF = mybir.ActivationFunctionType

    nc = bacc.Bacc(
        "TRN2", debug=False, target_bir_lowering=False, num_devices=NCORES
    )

    f8 = mybir.dt.float8e4
    xT = nc.dram_tensor("xT", [128, NQC * NDC * SC], bf16, kind="ExternalInput").ap()
    # fp8 DoubleRow operands for the Q/K projections: K=256 per matmul.
    # xq8 cols per quarter/chunk: (512 j) x (2 r) interleaved, r innermost;
    # wqk8 cols per block/chunk: (2 r) x (128 m), m innermost. W scaled by 64
    # (fp8e4m3 normal range); the 1/(64*64) is folded into the exp scale.
    xq8 = nc.dram_tensor(
        "xq8", [128, NQC * 4 * 1024], f8, kind="ExternalInput"
    ).ap()
    wqk8 = nc.dram_tensor(
        "wqk8", [128, 4 * 4 * 256], f8, kind="ExternalInput"
    ).ap()
    wv = nc.dram_tensor("wv", [128, NDC * 256], bf16, kind="ExternalInput").ap()
    wo = nc.dram_tensor("wo", [128, NPAIR * D], bf16, kind="ExternalInput").ap()
    # tri: cols 0:128 = identity, cols 128:256 = strictly-lower tril(-1e30)
    tri = nc.dram_tensor("tri", [128, 256], bf16, kind="ExternalInput").ap()
    out = nc.dram_tensor("out", [S, D], f32, kind="ExternalOutput").ap()

    with tile.TileContext(nc) as tc, ExitStack() as ctx:
        persist = ctx.enter_context(tc.tile_pool(name="persist", bufs=1))
        pt_pool = ctx.enter_context(tc.tile_pool(name="pt", bufs=8))
        den_pool = ctx.enter_context(tc.tile_pool(name="den", bufs=4))
        out_pool = ctx.enter_context(tc.tile_pool(name="outsb", bufs=2))
        ps_pool = ctx.enter_context(tc.tile_pool(name="ps", bufs=2, space="PSUM"))
        pa_pool = ctx.enter_context(tc.tile_pool(name="pa", bufs=2, space="PSUM"))

        # ---- persistent SBUF tensors (per s-quarter where it matters) ----
        x_sb = {
            q: persist.tile([128, NDC * SC], bf16, tag=f"x{q}", name=f"x{q}")
            for q in range(1, NQC)
        }
        x0_sb = [
            persist.tile([128, SC], bf16, tag=f"x0_{dc}", name=f"x0_{dc}")
            for dc in range(NDC)
        ]

        def x_slice(q, dc, lo=0, hi=SC):
            if q == 0:
                return x0_sb[dc][:, lo:hi]
            return x_sb[q][:, dc * SC + lo : dc * SC + hi]

        xq8_sb = persist.tile(
            [128, NQC * 4 * 1024], f8, tag="xq8", name="xq8_sb"
        )
        wqk8_sb = persist.tile([128, 4 * 4 * 256], f8, tag="wqk8", name="wqk8_sb")
        wv_sb = persist.tile([128, NDC * 256], bf16, tag="wv", name="wv_sb")
        wo_sb = persist.tile([128, NPAIR * D], bf16, tag="wo", name="wo_sb")
        trib_sb = persist.tile([128, 256], bf16, tag="trib", name="trib_sb")
        ones_sb = persist.tile([128, 1], f32, tag="ones", name="ones_sb")
        qt_sb = {
            (p, q): persist.tile([128, SC], bf16, tag=f"qt{p}_{q}", name=f"qt{p}_{q}")
            for p in range(NPAIR)
            for q in range(NQC)
        }
        kt_sb = {
            (p, q): persist.tile([128, SC], bf16, tag=f"kt{p}_{q}", name=f"kt{p}_{q}")
            for p in range(NPAIR)
            for q in range(NQC)
        }
        vo_sb = {
            q: persist.tile(
                [128, HPC * VO_QSTRIDE], bf16, tag=f"vo{q}", name=f"vo{q}"
            )
            for q in range(NQC)
        }
        # per-pair denominator rows (par segments side by side), all at
        # partition 0 so partition_broadcast needs no bounce copy
        den1 = [
            persist.tile([1, 2 * SC], f32, tag=f"den1_{p}", name=f"den1_{p}")
            for p in range(NPAIR)
        ]
        denr1 = [
            persist.tile([1, 2 * SC], f32, tag=f"denr1_{p}", name=f"denr1_{p}")
            for p in range(NPAIR)
        ]
        warm_sb = persist.tile([128, SC], bf16, tag="warm", name="warm_sb")
        at_sb = {
            (p, qc): persist.tile(
                [128, SC], bf16, tag=f"at{p}_{qc}", name=f"at{p}_{qc}"
            )
            for p in range(NPAIR)
            for qc in range(NQC)
        }

        # ---- loads: first-matmul dependencies first (sync HWDGE is FIFO);
        # tri right after the first QK operands - window 0 is all-diagonal,
        # so its exps depend on the mask tile ----
        nc.sync.dma_start(wqk8_sb[:], wqk8[:])
        nc.sync.dma_start(xq8_sb[:, 0:4096], xq8[:, 0:4096])
        nc.sync.dma_start(trib_sb[:], tri[:])
        for dc in range(NDC):
            nc.sync.dma_start(x0_sb[dc][:], xT[:, dc * SC : (dc + 1) * SC])
        nc.sync.dma_start(wv_sb[:], wv[:])
        for q in range(1, NQC):
            nc.sync.dma_start(
                xq8_sb[:, q * 4096 : (q + 1) * 4096],
                xq8[:, q * 4096 : (q + 1) * 4096],
            )
        nc.sync.dma_start(x_sb[1][:], xT[:, NDC * SC : 2 * NDC * SC])
        nc.sync.dma_start(wo_sb[:], wo[:])
        for q in range(2, NQC):
            nc.sync.dma_start(x_sb[q][:], xT[:, q * NDC * SC : (q + 1) * NDC * SC])
        nc.vector.memset(warm_sb[:], 0.0)
        nc.vector.memset(ones_sb[:], 1.0)
        for p in range(NPAIR):
            nc.vector.memset(den1[p][:], 1.0)
        for q in range(NQC):
            ones_cols = vo_sb[q].rearrange(
                "p (h s w) -> p h s w", h=HPC, w=VO_W
            )[:, :, :, 64]
            nc.vector.tensor_copy(
                ones_cols, ones_sb[:].to_broadcast((128, HPC, 4))
            )

        # ---- HAM warmup: dummy matmuls during the initial DMA wait flip the
        # PE clock gate to 8/8 before the first real matmul arrives ----
        warm_ps = ps_pool.tile([128, SC], f32, tag="ps", name="warm_ps")
        for i in range(20):
            nc.tensor.matmul(
                warm_ps[:],
                lhsT=warm_sb[:, 0:128],
                rhs=warm_sb[:, 0:SC],
                start=(i == 0),
                stop=(i == 19),
            )
        # force the gpsimd partition_broadcast ucode library to load NOW
        # (during the initial DMA wait) - it is the only gpsimd library the
        # kernel uses, so no mid-kernel ~7us library swaps occur
        warmb_sb = persist.tile([64, 4], f32, tag="warmb", name="warmb_sb")
        nc.gpsimd.partition_broadcast(warmb_sb[:], den1[0][0:1, 0:4])


        # ---- QKV projection thunks ----
        def qk_chain(p, qk, q, on_dve=False):
            dst = qt_sb[(p, q)] if qk == 0 else kt_sb[(p, q)]
            ps = ps_pool.tile([128, SC], f32, tag="ps", name=f"psqk{p}{qk}{q}")
            blk = qk * NPAIR + p
            for c in range(4):
                col = (blk * 4 + c) * 256
                nc.tensor.matmul(
                    ps[:, 0:SC],
                    lhsT=wqk8_sb[:, col : col + 256].rearrange(
                        "p (r m) -> p r m", r=2
                    ),
                    rhs=xq8_sb[
                        :, (q * 4 + c) * 1024 : (q * 4 + c + 1) * 1024
                    ].rearrange("p (r j) -> p r j", r=2),
                    start=(c == 0),
                    stop=(c == 3),
                    perf_mode=mybir.MatmulPerfMode.DoubleRow,
                )
            if on_dve:
                nc.vector.tensor_copy(dst[:], ps[:, 0:SC])
            else:
                nc.scalar.copy(dst[:], ps[:, 0:SC])

        def v_group(q, st4, on_dve=False):
            ps = ps_pool.tile([128, 256], f32, tag="ps", name=f"psv{q}{st4}")
            for dc in range(NDC):
                nc.tensor.matmul(
                    ps[:],
                    lhsT=x_slice(q, dc, st4 * 128, (st4 + 1) * 128),
                    rhs=wv_sb[:, dc * 256 : (dc + 1) * 256],
                    start=(dc == 0),
                    stop=(dc == NDC - 1),
                )
            vo_cols = vo_sb[q].rearrange(
                "p (h s w) -> p h s w", h=HPC, w=VO_W
            )[:, :, st4, 0:64]
            src = ps[:].rearrange("p (h e) -> p h e", e=64)
            if on_dve:
                nc.vector.tensor_copy(vo_cols, src)
            else:
                nc.scalar.copy(vo_cols, src)

        def q_thunks(q):
            return [lambda p=p: qk_chain(p, 0, q) for p in range(NPAIR)]

        def k_thunks(q, on_dve=False):
            return [lambda p=p: qk_chain(p, 1, q, on_dve) for p in range(NPAIR)]

        def v_thunks(q, on_dve=False):
            return [
                lambda st4=st4: v_group(q, st4, on_dve) for st4 in range(4)
            ]

        # ---- W_O projection thunks: one PSUM slot each ----
        outt_tiles = {}

        def wo_half(qc, qt, dc):
            if dc == 0:
                outt_tiles[(qc, qt)] = out_pool.tile(
                    [128, D], f32, tag="outsb", name=f"o{qc}{qt}"
                )
            outt = outt_tiles[(qc, qt)]
            po = ps_pool.tile([128, SC], f32, tag="ps", name=f"po{qc}{qt}{dc}")
            for p in range(NPAIR):
                nc.tensor.matmul(
                    po[:],
                    lhsT=at_sb[(p, qc)][:, qt * 128 : (qt + 1) * 128],
                    rhs=wo_sb[:, p * D + dc * SC : p * D + (dc + 1) * SC],
                    start=(p == 0),
                    stop=(p == NPAIR - 1),
                )
            nc.vector.tensor_copy(outt[:, dc * SC : (dc + 1) * SC], po[:])
            if dc == 1:
                row = (qc * 4 + qt) * 128
                nc.sync.dma_start(out[row : row + 128, :], outt[:])

        def wo_thunks(qc):
            return [
                lambda qt=qt, dc=dc: wo_half(qc, qt, dc)
                for qt in range(4)
                for dc in range(2)
            ]

        # ---- attention window with interleaved fill thunks ----
        def emit_attention(qc, fill=(), epi_qt=None):
            fill = list(fill)
            tail_fill = []
            if epi_qt is not None:
                fill, tail_fill = fill[:-4], fill[-4:]
            popped = 0
            nkt = 4 * (qc + 1)
            pa_qc = {
                p: pa_pool.tile([VO_W, 2 * SC], f32, tag="pa", name=f"pa{qc}{p}")
                for p in range(NPAIR)
            }

            def flush(p, kt, ptile):
                j0 = max(0, kt * 128 - qc * SC)
                kq, kst = kt // 4, kt % 4
                for par in range(2):
                    hh = 2 * p + par
                    vbase = hh * VO_QSTRIDE + kst * VO_W
                    nc.tensor.matmul(
                        pa_qc[p][:, par * SC + j0 : (par + 1) * SC],
                        lhsT=vo_sb[kq][:, vbase : vbase + VO_W],
                        rhs=ptile[:, par * SC + j0 : (par + 1) * SC],
                        start=(kt == 0),
                        stop=(kt == nkt - 1),
                    )
                if kt == nkt - 1:
                    # pa(p) complete: extract (ACT) + invert (DVE) this pair's
                    # softmax denominators right away so normalize starts early
                    for par in range(2):
                        seg = slice(par * SC, (par + 1) * SC)
                        nc.scalar.copy(
                            den1[p][0:1, seg],
                            pa_qc[p][64:65, par * SC : (par + 1) * SC],
                        )
                        nc.vector.reciprocal_approx_fast(
                            denr1[p][0:1, seg], den1[p][0:1, seg]
                        )

            pending = []  # (p, kt, ptile) awaiting the P@V matmul
            # spread fills into the drain phase; the final window reserves
            # extra drain-region fills so the PE stays HAM-warm through the
            # normalize chain and into W_O(3)
            nsteps = nkt + (7 if qc == NQC - 1 else 3)
            step = 0

            def pop_fills():
                nonlocal popped
                # front-biased pacing: in-window deadlines (K/V of this very
                # window) sit at the head of the fill list
                want = min(len(fill), -(-len(fill) * (step + 1) // nsteps))
                while popped < want:
                    fill[popped]()
                    popped += 1

            for kt in range(nkt):
                j0 = max(0, kt * 128 - qc * SC)
                kq, kst = kt // 4, kt % 4
                diag = kt * 128 >= qc * SC
                # scores for both pairs first (64-row array tiling: the two
                # par heads run CONCURRENTLY on separate array halves), then
                # the full-array mask matmuls, so the PE switches tiling mode
                # at most twice per kt step
                ps_kt = {}
                for p in range(NPAIR):
                    ps_s = ps_kt[p] = ps_pool.tile(
                        [128, 2 * SC], f32, tag="ps", name=f"pss{qc}{p}{kt}"
                    )
                    for par in range(2):
                        nc.tensor.matmul(
                            ps_s[:, par * SC + j0 : (par + 1) * SC],
                            lhsT=kt_sb[(p, kq)][
                                par * 64 : (par + 1) * 64,
                                kst * 128 : (kst + 1) * 128,
                            ],
                            rhs=qt_sb[(p, qc)][par * 64 : (par + 1) * 64, j0:SC],
                            start=True,
                            stop=not diag,
                            tile_position=(64 * par, 0),
                        )
                for p in range(NPAIR):
                    ps_s = ps_kt[p]
                    if diag:
                        # causal mask on the diagonal 128-block: accumulate
                        # -1e30 strictly-below-diagonal via identity @ mask,
                        # both par blocks in ONE matmul (broadcast rhs)
                        nc.tensor.matmul(
                            ps_s.rearrange("p (b n) -> p b n", b=2)[
                                :, :, j0 : j0 + 128
                            ],
                            lhsT=trib_sb[:, 0:128],
                            rhs=trib_sb[:, 128:256]
                            .unsqueeze(1)
                            .to_broadcast((128, 2, 128)),
                            start=False,
                            stop=True,
                        )
                    ptile = pt_pool.tile(
                        [128, 2 * SC], bf16, tag="pt", name=f"pt{qc}{p}{kt}"
                    )
                    nc.scalar.activation(
                        ptile.rearrange("p (b n) -> p b n", b=2)[:, :, j0:SC],
                        ps_s.rearrange("p (b n) -> p b n", b=2)[:, :, j0:SC],
                        AF.Exp,
                        scale=0.125 / 4096,
                    )
                    pending.append((p, kt, ptile))
                while len(pending) > 6:
                    flush(*pending.pop(0))
                step = kt + 1
                pop_fills()
            for pend in pending:
                flush(*pend)
                step += 1
                pop_fills()
            # normalize BEFORE the trailing fills so its DVE/GpSimd chain
            # isn't queued behind their PSUM-evacuation copies
            _normalize(qc, pa_qc, epi_qt, tail_fill)
            step = nsteps
            pop_fills()

        def _normalize(qc, pa_qc, epi_qt=None, tail_fill=()):
            tail_fill = list(tail_fill)
            denb = {}
            for p in range(NPAIR):
                denb[p] = den_pool.tile(
                    [64, 2 * SC], f32, tag="denb", name=f"denb{qc}{p}"
                )
                nc.gpsimd.partition_broadcast(denb[p][:], denr1[p][:])
            if epi_qt is None:
                for p in range(NPAIR):
                    for par in range(2):
                        nc.vector.tensor_mul(
                            at_sb[(p, qc)][par * 64 : (par + 1) * 64, :],
                            pa_qc[p][0:64, par * SC : (par + 1) * SC],
                            denb[p][:, par * SC : (par + 1) * SC],
                        )
            else:
                # final window: per-qt muls so each W_O qt-block can start as
                # soon as its 128 q-columns are normalized; reserved fill
                # thunks interleave to keep the PE HAM-warm through the chain
                for qt in range(4):
                    if qt < len(tail_fill):
                        tail_fill[qt]()
                    c0, c1 = qt * 128, (qt + 1) * 128
                    for p in range(NPAIR):
                        for par in range(2):
                            nc.vector.tensor_mul(
                                at_sb[(p, qc)][par * 64 : (par + 1) * 64, c0:c1],
                                pa_qc[p][0:64, par * SC + c0 : par * SC + c1],
                                denb[p][:, par * SC + c0 : par * SC + c1],
                            )
                    epi_qt(qt)

        # ---- main schedule: prologue QK(0) pair-major (scores(p0, kt0) can
        # start after just two chains), then interleaved windows ----
        for p in range(NPAIR):
            qk_chain(p, 0, 0)
            qk_chain(p, 1, 0)
        # fills keyed by DEADLINE: only Q(q+1) must land in window q; K(q) and
        # V(q) pop early inside window q itself (scores kq=q start at kt=4q,
        # flushes trail); W_O(qc') goes anywhere after window qc'. This keeps
        # early windows light and gives the exp-bound late windows PE work.
        fills = {
            0: v_thunks(0, on_dve=True) + q_thunks(1),
            1: k_thunks(1, on_dve=True) + v_thunks(1, on_dve=True)
            + q_thunks(2) + wo_thunks(0),
            2: k_thunks(2, on_dve=True) + v_thunks(2, on_dve=True)
            + q_thunks(3) + wo_thunks(1),
            3: k_thunks(3, on_dve=True) + v_thunks(3, on_dve=True)
            + wo_thunks(2),
        }

        def _wo3_epi(qt):
            wo_half(3, qt, 0)
            wo_half(3, qt, 1)

        for qc in range(NQC):
            emit_attention(
                qc, fill=fills[qc], epi_qt=_wo3_epi if qc == NQC - 1 else None
            )

    nc.compile()
    return nc


def _get_program():
    if "nc" not in _cache:
        _cache["nc"] = _build_program()
    return _cache["nc"]


def _prep_core_inputs(c, residual, W_Q, W_K, W_V, W_O, tri):
    import ml_dtypes

    b = c // 4
    heads = [4 * (c % 4) + i for i in range(HPC)]

    def chunked(w):  # [1024, M] -> [128, NDC*M] chunk-major
        m = w.shape[1]
        return np.ascontiguousarray(
            w.reshape(NDC, 128, m).transpose(1, 0, 2).reshape(128, NDC * m)
        )

    # fp8 DoubleRow wqk blocks: [k, c, r, m] with m innermost, W scaled by 64
    wqk_blocks = []
    for Wt in (W_Q, W_K):
        for p in range(NPAIR):
            h0, h1 = heads[2 * p], heads[2 * p + 1]
            wpair = np.concatenate([Wt[h0].T, Wt[h1].T], axis=1)  # [1024, 128]
            blk = (
                (wpair * 64.0)
                .reshape(4, 2, 128, 128)
                .transpose(2, 0, 1, 3)
                .reshape(128, 4 * 2 * 128)
            )
            wqk_blocks.append(blk)
    wqk8_arr = np.ascontiguousarray(np.concatenate(wqk_blocks, axis=1)).astype(
        ml_dtypes.float8_e4m3
    )

    wv_arr = chunked(np.concatenate([W_V[h].T for h in heads], axis=1))
    wo_arr = np.ascontiguousarray(
        np.concatenate(
            [
                np.concatenate([W_O[heads[2 * p]], W_O[heads[2 * p + 1]]], axis=0)
                for p in range(NPAIR)
            ],
            axis=1,
        )
    )
    xtf = residual[b].T.astype(np.float32)  # [1024, 2048]
    xt = xtf.astype(ml_dtypes.bfloat16)
    xq = np.concatenate(
        [
            np.concatenate(
                [xt[dc * 128 : (dc + 1) * 128, q * SC : (q + 1) * SC]
                 for dc in range(NDC)], axis=1)
            for q in range(NQC)
        ],
        axis=1,
    )
    # fp8 x for DoubleRow QK: [k, q, c, r, j] - K-half r in the middle,
    # j innermost (CoreSim DoubleRow indexes both operands as [p, r, ...])
    xq8_arr = (
        xtf.reshape(4, 2, 128, 4, 512)
        .transpose(2, 3, 0, 1, 4)
        .reshape(128, NQC * 4 * 1024)
        .astype(ml_dtypes.float8_e4m3)
    )
    return {
        "xT": np.ascontiguousarray(xq),
        "xq8": np.ascontiguousarray(xq8_arr),
        "wqk8": wqk8_arr,
        "wv": wv_arr.astype(ml_dtypes.bfloat16),
        "wo": wo_arr.astype(ml_dtypes.bfloat16),
        "tri": tri,
    }


def make_in_maps(residual, W_Q, W_K, W_V, W_O):
    residual = np.asarray(residual, np.float32)
    W_Q, W_K, W_V, W_O = (np.asarray(w, np.float32) for w in (W_Q, W_K, W_V, W_O))
    import ml_dtypes

    # additive causal mask for S^T[k, q] diagonal blocks, applied on the PE:
    # identity (stationary) @ tril(-1e30, -1) accumulated onto the scores
    eye = np.eye(128, dtype=np.float32)
    neg = np.tril(np.full((128, 128), -1e30, np.float32), -1)
    tri = np.concatenate([eye, neg], axis=1).astype(ml_dtypes.bfloat16)
    return [
        _prep_core_inputs(c, residual, W_Q, W_K, W_V, W_O, tri)
        for c in range(NCORES)
    ]


def gather(results):
    out = np.zeros((B, S, D), np.float64)
    for c in range(NCORES):
        out[c // 4] += results[c]["out"].astype(np.float64)
    return out.astype(np.float32)


def kernel(residual, W_Q, W_K, W_V, W_O, **run_kwargs):
    from concourse.bass_utils import run_bass_kernel_spmd

    nc = _get_program()
    in_maps = make_in_maps(residual, W_Q, W_K, W_V, W_O)
    res = run_bass_kernel_spmd(nc, in_maps, core_ids=list(range(NCORES)), **run_kwargs)
    out = gather(res.results)
    if run_kwargs:
        _cache["last_results"] = res
    return out


# revision 56
# speedup vs baseline: 1.0166x; 1.0166x over previous
"""Causal multi-head attention (B=2, S=2048, D=1024, H=16, Dh=64) on 8 trn2 cores.

Sharding: (batch, head-group) tensor parallel. Core c handles batch c//4 and
heads [4*(c%4), 4*(c%4)+4). Each core computes its 4 heads end-to-end
(QKV projections, causal softmax attention, W_O projection) and returns a
partial [S, D] output; the host sums the 4 partials per batch.

Per-core dataflow (v2 - cross-phase software pipeline):
  - Q^T, K^T produced in [Dh, S] layout so scores come out transposed
    (S^T[k, q]) and the softmax'd P~ needs no transpose for the P@V matmul.
  - Softmax denominator via a ones-column appended to V (M=65 matmuls):
    row 64 of the attention PSUM is the denominator.
  - Causal mask: multiplicative upper-triangular bf16 tile applied to the
    exp'd diagonal blocks on GpSimd.
  - ACT exp (1 elem/lane/cycle @1.2GHz, ~293ns/inst) is the intrinsic
    attention bottleneck and the per-chunk exp load grows with qc (causal
    trapezoid). v1 serialized "QKV(q); attention(q)" so the PE starved
    behind exp via PSUM-slot reuse, HAM re-throttled to 1.2GHz repeatedly.
  - v2 interleaves emission: attention(qc) kt-steps pop "fill" thunks
    between steps (QKV(qc+1) projection chains, V|ones groups, W_O(qc-2)
    output-projection pieces), so the PE always has slot-independent
    matmul work while ACT exps, and windows approach per-engine balance:
      window qc0: fills = V(0) groups + full QKV(1)
      window qc1: fills = QKV(2) + W_O(0)
      window qc2: fills = QKV(3) + W_O(1)   (QKV(3) PSUM copies on DVE)
      window qc3: fills = W_O(2)
  - W_O thunks hold ONE PSUM slot each (per 512-col half of out rows) so
    they never block the scores double-buffer.
  - PSUM->SBUF copy engine per window budget: exp owns ACT during heavy
    windows, so QKV(3) copies + softmax-denominator chain go to DVE;
    QKV(0..2) copies stay on ACT (spare there).
  - bf16 operands everywhere on the PE (fast weight load); fp32 PSUM
    accumulation; W_O stationary in float32r; softmax scale /8 inside the
    exp's free affine.
"""

import numpy as np

try:
    import concourse  # noqa: F401
except ImportError:  # pragma: no cover - harness containers stage it here
    import sys

    sys.path.insert(0, "/opt/trn_rl_repo")

B, S, D, H, DH = 2, 2048, 1024, 16, 64
NCORES = 8
HPC = 4  # heads per core
NPAIR = 2  # head pairs per core
SC = 512  # q-chunk width (scores matmul N)
NQC = S // SC  # 4 q-chunks
NST = S // 128  # 16 s/k/q tiles of 128
NDC = D // 128  # 8 contraction chunks of 128
VO_W = 65  # V columns + ones column
VO_QSTRIDE = 4 * VO_W  # per-head stride inside one quarter's V|ones tile

_cache = {}


def _build_program():
    from contextlib import ExitStack

    import concourse.mybir as mybir
    import concourse.tile as tile
    from concourse import bacc

    f32 = mybir.dt.float32
    f32r = mybir.dt.float32r
    bf16 = mybir.dt.bfloat16
    AF = mybir.ActivationFunctionType

    nc = bacc.Bacc(
        "TRN2", debug=False, target_bir_lowering=False, num_devices=NCORES
    )

    f8 = mybir.dt.float8e4
    xT = nc.dram_tensor("xT", [128, NQC * NDC * SC], bf16, kind="ExternalInput").ap()
    # fp8 DoubleRow operands for the Q/K projections: K=256 per matmul.
    # xq8 cols per quarter/chunk: (512 j) x (2 r) interleaved, r innermost;
    # wqk8 cols per block/chunk: (2 r) x (128 m), m innermost. W scaled by 64
    # (fp8e4m3 normal range); the 1/(64*64) is folded into the exp scale.
    xq8 = nc.dram_tensor(
        "xq8", [128, NQC * 4 * 1024], f8, kind="ExternalInput"
    ).ap()
    wqk8 = nc.dram_tensor(
        "wqk8", [128, 4 * 4 * 256], f8, kind="ExternalInput"
    ).ap()
    wv = nc.dram_tensor("wv", [128, NDC * 256], bf16, kind="ExternalInput").ap()
    wo = nc.dram_tensor("wo", [128, NPAIR * D], bf16, kind="ExternalInput").ap()
    # tri: cols 0:128 = identity, cols 128:256 = strictly-lower tril(-1e30)
    tri = nc.dram_tensor("tri", [128, 256], bf16, kind="ExternalInput").ap()
    out = nc.dram_tensor("out", [S, D], f32, kind="ExternalOutput").ap()

    with tile.TileContext(nc) as tc, ExitStack() as ctx:
        persist = ctx.enter_context(tc.tile_pool(name="persist", bufs=1))
        pt_pool = ctx.enter_context(tc.tile_pool(name="pt", bufs=8))
        den_pool = ctx.enter_context(tc.tile_pool(name="den", bufs=4))
        out_pool = ctx.enter_context(tc.tile_pool(name="outsb", bufs=2))
        ps_pool = ctx.enter_context(tc.tile_pool(name="ps", bufs=2, space="PSUM"))
        pa_pool = ctx.enter_context(tc.tile_pool(name="pa", bufs=2, space="PSUM"))

        # ---- persistent SBUF tensors (per s-quarter where it matters) ----
        x_sb = {
            q: persist.tile([128, NDC * SC], bf16, tag=f"x{q}", name=f"x{q}")
            for q in range(1, NQC)
        }
        x0_sb = [
            persist.tile([128, SC], bf16, tag=f"x0_{dc}", name=f"x0_{dc}")
            for dc in range(NDC)
        ]

        def x_slice(q, dc, lo=0, hi=SC):
            if q == 0:
                return x0_sb[dc][:, lo:hi]
            return x_sb[q][:, dc * SC + lo : dc * SC + hi]

        xq8_sb = persist.tile(
            [128, NQC * 4 * 1024], f8, tag="xq8", name="xq8_sb"
        )
        wqk8_sb = persist.tile([128, 4 * 4 * 256], f8, tag="wqk8", name="wqk8_sb")
        wv_sb = persist.tile([128, NDC * 256], bf16, tag="wv", name="wv_sb")
        wo_sb = persist.tile([128, NPAIR * D], bf16, tag="wo", name="wo_sb")
        trib_sb = persist.tile([128, 256], bf16, tag="trib", name="trib_sb")
        ones_sb = persist.tile([128, 1], f32, tag="ones", name="ones_sb")
        qt_sb = {
            (p, q): persist.tile([128, SC], bf16, tag=f"qt{p}_{q}", name=f"qt{p}_{q}")
            for p in range(NPAIR)
            for q in range(NQC)
        }
        kt_sb = {
            (p, q): persist.tile([128, SC], bf16, tag=f"kt{p}_{q}", name=f"kt{p}_{q}")
            for p in range(NPAIR)
            for q in range(NQC)
        }
        vo_sb = {
            q: persist.tile(
                [128, HPC * VO_QSTRIDE], bf16, tag=f"vo{q}", name=f"vo{q}"
            )
            for q in range(NQC)
        }
        # per-pair denominator rows (par segments side by side), all at
        # partition 0 so partition_broadcast needs no bounce copy
        den1 = [
            persist.tile([1, 2 * SC], f32, tag=f"den1_{p}", name=f"den1_{p}")
            for p in range(NPAIR)
        ]
        denr1 = [
            persist.tile([1, 2 * SC], f32, tag=f"denr1_{p}", name=f"denr1_{p}")
            for p in range(NPAIR)
        ]
        warm_sb = persist.tile([128, SC], bf16, tag="warm", name="warm_sb")
        at_sb = {
            (p, qc): persist.tile(
                [128, SC], bf16, tag=f"at{p}_{qc}", name=f"at{p}_{qc}"
            )
            for p in range(NPAIR)
            for qc in range(NQC)
        }

        # ---- loads: first-matmul dependencies first (sync HWDGE is FIFO);
        # tri right after the first QK operands - window 0 is all-diagonal,
        # so its exps depend on the mask tile ----
        nc.sync.dma_start(wqk8_sb[:], wqk8[:])
        nc.sync.dma_start(xq8_sb[:, 0:4096], xq8[:, 0:4096])
        nc.sync.dma_start(trib_sb[:], tri[:])
        for dc in range(NDC):
            nc.sync.dma_start(x0_sb[dc][:], xT[:, dc * SC : (dc + 1) * SC])
        nc.sync.dma_start(wv_sb[:], wv[:])
        for q in range(1, NQC):
            nc.sync.dma_start(
                xq8_sb[:, q * 4096 : (q + 1) * 4096],
                xq8[:, q * 4096 : (q + 1) * 4096],
            )
        nc.sync.dma_start(x_sb[1][:], xT[:, NDC * SC : 2 * NDC * SC])
        nc.sync.dma_start(wo_sb[:], wo[:])
        for q in range(2, NQC):
            nc.sync.dma_start(x_sb[q][:], xT[:, q * NDC * SC : (q + 1) * NDC * SC])
        nc.vector.memset(warm_sb[:], 0.0)
        nc.vector.memset(ones_sb[:], 1.0)
        for p in range(NPAIR):
            nc.vector.memset(den1[p][:], 1.0)
        for q in range(NQC):
            ones_cols = vo_sb[q].rearrange(
                "p (h s w) -> p h s w", h=HPC, w=VO_W
            )[:, :, :, 64]
            nc.vector.tensor_copy(
                ones_cols, ones_sb[:].to_broadcast((128, HPC, 4))
            )

        # ---- HAM warmup: dummy matmuls during the initial DMA wait flip the
        # PE clock gate to 8/8 before the first real matmul arrives ----
        warm_ps = ps_pool.tile([128, SC], f32, tag="ps", name="warm_ps")
        for i in range(20):
            nc.tensor.matmul(
                warm_ps[:],
                lhsT=warm_sb[:, 0:128],
                rhs=warm_sb[:, 0:SC],
                start=(i == 0),
                stop=(i == 19),
            )
        # force the gpsimd partition_broadcast ucode library to load NOW
        # (during the initial DMA wait) - it is the only gpsimd library the
        # kernel uses, so no mid-kernel ~7us library swaps occur
        warmb_sb = persist.tile([64, 4], f32, tag="warmb", name="warmb_sb")
        nc.gpsimd.partition_broadcast(warmb_sb[:], den1[0][0:1, 0:4])


        # ---- QKV projection thunks ----
        def qk_chain(p, qk, q, on_dve=False):
            dst = qt_sb[(p, q)] if qk == 0 else kt_sb[(p, q)]
            ps = ps_pool.tile([128, SC], f32, tag="ps", name=f"psqk{p}{qk}{q}")
            blk = qk * NPAIR + p
            for c in range(4):
                col = (blk * 4 + c) * 256
                nc.tensor.matmul(
                    ps[:, 0:SC],
                    lhsT=wqk8_sb[:, col : col + 256].rearrange(
                        "p (r m) -> p r m", r=2
                    ),
                    rhs=xq8_sb[
                        :, (q * 4 + c) * 1024 : (q * 4 + c + 1) * 1024
                    ].rearrange("p (r j) -> p r j", r=2),
                    start=(c == 0),
                    stop=(c == 3),
                    perf_mode=mybir.MatmulPerfMode.DoubleRow,
                )
            if on_dve:
                nc.vector.tensor_copy(dst[:], ps[:, 0:SC])
            else:
                nc.scalar.copy(dst[:], ps[:, 0:SC])

        def v_group(q, st4, on_dve=False):
            ps = ps_pool.tile([128, 256], f32, tag="ps", name=f"psv{q}{st4}")
            for dc in range(NDC):
                nc.tensor.matmul(
                    ps[:],
                    lhsT=x_slice(q, dc, st4 * 128, (st4 + 1) * 128),
                    rhs=wv_sb[:, dc * 256 : (dc + 1) * 256],
                    start=(dc == 0),
                    stop=(dc == NDC - 1),
                )
            vo_cols = vo_sb[q].rearrange(
                "p (h s w) -> p h s w", h=HPC, w=VO_W
            )[:, :, st4, 0:64]
            src = ps[:].rearrange("p (h e) -> p h e", e=64)
            if on_dve:
                nc.vector.tensor_copy(vo_cols, src)
            else:
                nc.scalar.copy(vo_cols, src)

        def q_thunks(q):
            return [lambda p=p: qk_chain(p, 0, q) for p in range(NPAIR)]

        def k_thunks(q, on_dve=False):
            return [lambda p=p: qk_chain(p, 1, q, on_dve) for p in range(NPAIR)]

        def v_thunks(q, on_dve=False):
            return [
                lambda st4=st4: v_group(q, st4, on_dve) for st4 in range(4)
            ]

        # ---- W_O projection thunks: one PSUM slot each ----
        outt_tiles = {}

        def wo_half(qc, qt, dc):
            if dc == 0:
                outt_tiles[(qc, qt)] = out_pool.tile(
                    [128, D], f32, tag="outsb", name=f"o{qc}{qt}"
                )
            outt = outt_tiles[(qc, qt)]
            po = ps_pool.tile([128, SC], f32, tag="ps", name=f"po{qc}{qt}{dc}")
            for p in range(NPAIR):
                nc.tensor.matmul(
                    po[:],
                    lhsT=at_sb[(p, qc)][:, qt * 128 : (qt + 1) * 128],
                    rhs=wo_sb[:, p * D + dc * SC : p * D + (dc + 1) * SC],
                    start=(p == 0),
                    stop=(p == NPAIR - 1),
                )
            nc.vector.tensor_copy(outt[:, dc * SC : (dc + 1) * SC], po[:])
            if dc == 1:
                row = (qc * 4 + qt) * 128
                nc.sync.dma_start(out[row : row + 128, :], outt[:])

        def wo_thunks(qc):
            return [
                lambda qt=qt, dc=dc: wo_half(qc, qt, dc)
                for qt in range(4)
                for dc in range(2)
            ]

        # ---- attention window with interleaved fill thunks ----
        def emit_attention(qc, fill=(), epi_qt=None):
            fill = list(fill)
            tail_fill = []
            if epi_qt is not None:
                fill, tail_fill = fill[:-4], fill[-4:]
            popped = 0
            nkt = 4 * (qc + 1)
            pa_qc = {
                p: pa_pool.tile([VO_W, 2 * SC], f32, tag="pa", name=f"pa{qc}{p}")
                for p in range(NPAIR)
            }

            def flush(p, kt, ptile):
                j0 = max(0, kt * 128 - qc * SC)
                kq, kst = kt // 4, kt % 4
                for par in range(2):
                    hh = 2 * p + par
                    vbase = hh * VO_QSTRIDE + kst * VO_W
                    nc.tensor.matmul(
                        pa_qc[p][:, par * SC + j0 : (par + 1) * SC],
                        lhsT=vo_sb[kq][:, vbase : vbase + VO_W],
                        rhs=ptile[:, par * SC + j0 : (par + 1) * SC],
                        start=(kt == 0),
                        stop=(kt == nkt - 1),
                    )
                if kt == nkt - 1:
                    # pa(p) complete: extract (ACT) + invert (DVE) this pair's
                    # softmax denominators right away so normalize starts early
                    for par in range(2):
                        seg = slice(par * SC, (par + 1) * SC)
                        nc.scalar.copy(
                            den1[p][0:1, seg],
                            pa_qc[p][64:65, par * SC : (par + 1) * SC],
                        )
                        nc.vector.reciprocal_approx_fast(
                            denr1[p][0:1, seg], den1[p][0:1, seg]
                        )

            pending = []  # (p, kt, ptile) awaiting the P@V matmul
            # spread fills into the drain phase; the final window reserves
            # extra drain-region fills so the PE stays HAM-warm through the
            # normalize chain and into W_O(3)
            nsteps = nkt + (7 if qc == NQC - 1 else 3)
            step = 0

            def pop_fills():
                nonlocal popped
                # front-biased pacing: in-window deadlines (K/V of this very
                # window) sit at the head of the fill list
                want = min(len(fill), -(-len(fill) * (step + 1) // nsteps))
                while popped < want:
                    fill[popped]()
                    popped += 1

            for kt in range(nkt):
                j0 = max(0, kt * 128 - qc * SC)
                kq, kst = kt // 4, kt % 4
                diag = kt * 128 >= qc * SC
                # scores for both pairs first (64-row array tiling: the two
                # par heads run CONCURRENTLY on separate array halves), then
                # the full-array mask matmuls, so the PE switches tiling mode
                # at most twice per kt step
                ps_kt = {}
                for p in range(NPAIR):
                    ps_s = ps_kt[p] = ps_pool.tile(
                        [128, 2 * SC], f32, tag="ps", name=f"pss{qc}{p}{kt}"
                    )
                    for par in range(2):
                        nc.tensor.matmul(
                            ps_s[:, par * SC + j0 : (par + 1) * SC],
                            lhsT=kt_sb[(p, kq)][
                                par * 64 : (par + 1) * 64,
                                kst * 128 : (kst + 1) * 128,
                            ],
                            rhs=qt_sb[(p, qc)][par * 64 : (par + 1) * 64, j0:SC],
                            start=True,
                            stop=not diag,
                            tile_position=(64 * par, 0),
                        )
                for p in range(NPAIR):
                    ps_s = ps_kt[p]
                    if diag:
                        # causal mask on the diagonal 128-block: accumulate
                        # -1e30 strictly-below-diagonal via identity @ mask
                        # (one matmul per par: a PSUM write can't cross banks)
                        for par in range(2):
                            nc.tensor.matmul(
                                ps_s[:, par * SC + j0 : par * SC + j0 + 128],
                                lhsT=trib_sb[:, 0:128],
                                rhs=trib_sb[:, 128:256],
                                start=False,
                                stop=True,
                            )
                    ptile = pt_pool.tile(
                        [128, 2 * SC], bf16, tag="pt", name=f"pt{qc}{p}{kt}"
                    )
                    nc.scalar.activation(
                        ptile.rearrange("p (b n) -> p b n", b=2)[:, :, j0:SC],
                        ps_s.rearrange("p (b n) -> p b n", b=2)[:, :, j0:SC],
                        AF.Exp,
                        scale=0.125 / 4096,
                    )
                    pending.append((p, kt, ptile))
                while len(pending) > 6:
                    flush(*pending.pop(0))
                step = kt + 1
                pop_fills()
            for pend in pending:
                flush(*pend)
                step += 1
                pop_fills()
            # normalize BEFORE the trailing fills so its DVE/GpSimd chain
            # isn't queued behind their PSUM-evacuation copies
            _normalize(qc, pa_qc, epi_qt, tail_fill)
            step = nsteps
            pop_fills()

        def _normalize(qc, pa_qc, epi_qt=None, tail_fill=()):
            tail_fill = list(tail_fill)
            denb = {}
            for p in range(NPAIR):
                denb[p] = den_pool.tile(
                    [64, 2 * SC], f32, tag="denb", name=f"denb{qc}{p}"
                )
                nc.gpsimd.partition_broadcast(denb[p][:], denr1[p][:])
            if epi_qt is None:
                for p in range(NPAIR):
                    for par in range(2):
                        nc.vector.tensor_mul(
                            at_sb[(p, qc)][par * 64 : (par + 1) * 64, :],
                            pa_qc[p][0:64, par * SC : (par + 1) * SC],
                            denb[p][:, par * SC : (par + 1) * SC],
                        )
            else:
                # final window: per-qt muls so each W_O qt-block can start as
                # soon as its 128 q-columns are normalized; reserved fill
                # thunks interleave to keep the PE HAM-warm through the chain
                for qt in range(4):
                    if qt < len(tail_fill):
                        tail_fill[qt]()
                    c0, c1 = qt * 128, (qt + 1) * 128
                    for p in range(NPAIR):
                        for par in range(2):
                            nc.vector.tensor_mul(
                                at_sb[(p, qc)][par * 64 : (par + 1) * 64, c0:c1],
                                pa_qc[p][0:64, par * SC + c0 : par * SC + c1],
                                denb[p][:, par * SC + c0 : par * SC + c1],
                            )
                    epi_qt(qt)

        # ---- main schedule: prologue QK(0) pair-major (scores(p0, kt0) can
        # start after just two chains), then interleaved windows ----
        for p in range(NPAIR):
            qk_chain(p, 0, 0)
            qk_chain(p, 1, 0)
        # fills keyed by DEADLINE: only Q(q+1) must land in window q; K(q) and
        # V(q) pop early inside window q itself (scores kq=q start at kt=4q,
        # flushes trail); W_O(qc') goes anywhere after window qc'. This keeps
        # early windows light and gives the exp-bound late windows PE work.
        fills = {
            0: v_thunks(0, on_dve=True) + q_thunks(1),
            1: k_thunks(1, on_dve=True) + v_thunks(1, on_dve=True)
            + q_thunks(2) + wo_thunks(0),
            2: k_thunks(2, on_dve=True) + v_thunks(2, on_dve=True)
            + q_thunks(3) + wo_thunks(1),
            3: k_thunks(3, on_dve=True) + v_thunks(3, on_dve=True)
            + wo_thunks(2),
        }

        def _wo3_epi(qt):
            wo_half(3, qt, 0)
            wo_half(3, qt, 1)

        for qc in range(NQC):
            emit_attention(
                qc, fill=fills[qc], epi_qt=_wo3_epi if qc == NQC - 1 else None
            )

    nc.compile()
    return nc


def _get_program():
    if "nc" not in _cache:
        _cache["nc"] = _build_program()
    return _cache["nc"]


def _prep_core_inputs(c, residual, W_Q, W_K, W_V, W_O, tri):
    import ml_dtypes

    b = c // 4
    heads = [4 * (c % 4) + i for i in range(HPC)]

    def chunked(w):  # [1024, M] -> [128, NDC*M] chunk-major
        m = w.shape[1]
        return np.ascontiguousarray(
            w.reshape(NDC, 128, m).transpose(1, 0, 2).reshape(128, NDC * m)
        )

    # fp8 DoubleRow wqk blocks: [k, c, r, m] with m innermost, W scaled by 64
    wqk_blocks = []
    for Wt in (W_Q, W_K):
        for p in range(NPAIR):
            h0, h1 = heads[2 * p], heads[2 * p + 1]
            wpair = np.concatenate([Wt[h0].T, Wt[h1].T], axis=1)  # [1024, 128]
            blk = (
                (wpair * 64.0)
                .reshape(4, 2, 128, 128)
                .transpose(2, 0, 1, 3)
                .reshape(128, 4 * 2 * 128)
            )
            wqk_blocks.append(blk)
    wqk8_arr = np.ascontiguousarray(np.concatenate(wqk_blocks, axis=1)).astype(
        ml_dtypes.float8_e4m3
    )

    wv_arr = chunked(np.concatenate([W_V[h].T for h in heads], axis=1))
    wo_arr = np.ascontiguousarray(
        np.concatenate(
            [
                np.concatenate([W_O[heads[2 * p]], W_O[heads[2 * p + 1]]], axis=0)
                for p in range(NPAIR)
            ],
            axis=1,
        )
    )
    xtf = residual[b].T.astype(np.float32)  # [1024, 2048]
    xt = xtf.astype(ml_dtypes.bfloat16)
    xq = np.concatenate(
        [
            np.concatenate(
                [xt[dc * 128 : (dc + 1) * 128, q * SC : (q + 1) * SC]
                 for dc in range(NDC)], axis=1)
            for q in range(NQC)
        ],
        axis=1,
    )
    # fp8 x for DoubleRow QK: [k, q, c, r, j] - K-half r in the middle,
    # j innermost (CoreSim DoubleRow indexes both operands as [p, r, ...])
    xq8_arr = (
        xtf.reshape(4, 2, 128, 4, 512)
        .transpose(2, 3, 0, 1, 4)
        .reshape(128, NQC * 4 * 1024)
        .astype(ml_dtypes.float8_e4m3)
    )
    return {
        "xT": np.ascontiguousarray(xq),
        "xq8": np.ascontiguousarray(xq8_arr),
        "wqk8": wqk8_arr,
        "wv": wv_arr.astype(ml_dtypes.bfloat16),
        "wo": wo_arr.astype(ml_dtypes.bfloat16),
        "tri": tri,
    }


def make_in_maps(residual, W_Q, W_K, W_V, W_O):
    residual = np.asarray(residual, np.float32)
    W_Q, W_K, W_V, W_O = (np.asarray(w, np.float32) for w in (W_Q, W_K, W_V, W_O))
    import ml_dtypes

    # additive causal mask for S^T[k, q] diagonal blocks, applied on the PE:
    # identity (stationary) @ tril(-1e30, -1) accumulated onto the scores
    eye = np.eye(128, dtype=np.float32)
    neg = np.tril(np.full((128, 128), -1e30, np.float32), -1)
    tri = np.concatenate([eye, neg], axis=1).astype(ml_dtypes.bfloat16)
    return [
        _prep_core_inputs(c, residual, W_Q, W_K, W_V, W_O, tri)
        for c in range(NCORES)
    ]


def gather(results):
    out = np.zeros((B, S, D), np.float64)
    for c in range(NCORES):
        out[c // 4] += results[c]["out"].astype(np.float64)
    return out.astype(np.float32)


def kernel(residual, W_Q, W_K, W_V, W_O, **run_kwargs):
    from concourse.bass_utils import run_bass_kernel_spmd

    nc = _get_program()
    in_maps = make_in_maps(residual, W_Q, W_K, W_V, W_O)
    res = run_bass_kernel_spmd(nc, in_maps, core_ids=list(range(NCORES)), **run_kwargs)
    out = gather(res.results)
    if run_kwargs:
        _cache["last_results"] = res
    return out


# revision 59
# speedup vs baseline: 1.0392x; 1.0222x over previous
"""Causal multi-head attention (B=2, S=2048, D=1024, H=16, Dh=64) on 8 trn2 cores.

Sharding: (batch, head-group) tensor parallel. Core c handles batch c//4 and
heads [4*(c%4), 4*(c%4)+4). Each core computes its 4 heads end-to-end
(QKV projections, causal softmax attention, W_O projection) and returns a
partial [S, D] output; the host sums the 4 partials per batch.

Per-core dataflow (cross-phase software pipeline, one attention "window"
per 512-wide q-chunk qc; window qc's kt-steps pop "fill" thunks so the PE
always has slot-independent matmul work while ACT runs the exps):
  - Q^T, K^T produced in [Dh, S] layout so scores come out transposed
    (S^T[k, q]) and the softmax'd P~ needs no transpose for the P@V matmul.
  - Q/K projections: fp8e4m3 DoubleRow matmuls (K=256 per matmul, W scaled
    by 64 into fp8 normal range, the 1/4096 folded into the exp scale).
    V and W_O stay bf16 (their quantization error would pass straight
    through to the output; Q/K noise is softened by the softmax).
  - Scores: 64-row PE array tiling - the two par heads run concurrently on
    separate array halves (tile_position (0,0)/(64,0), different banks).
  - Causal mask: additive -1e30 accumulated onto the diagonal PSUM scores
    blocks via an identity @ mask matmul - the mask lives entirely on the
    PE, so GpSimd only ever runs partition_broadcast and its ucode library
    is loaded exactly once (a mid-kernel library swap stalls ~7us).
  - Softmax denominator via a ones-column appended to V (M=65 flush
    matmuls): row 64 of the attention PSUM is the denominator. Per-pair
    rows live at partition 0 ([1, 2*SC] tiles) so partition_broadcast
    needs no bounce copy; reciprocal_approx_fast (~18 bits) inverts them.
  - Fill scheduling is deadline-based: only Q(q+1) must land in window q;
    K(q)/V(q) pop early inside window q itself (scores for kq=q start at
    kt=4q and flushes trail by >6 kt); W_O(qc') fills any window > qc'.
    The final window runs per-qt normalize muls with the W_O(3) epilogue
    interleaved per qt-block.
  - HAM warmup: 20 dummy matmuls during the initial DMA dead time flip the
    PE clock gate to 8/8 before real work arrives. Load order puts the
    first window's operands (wqk8, xq8 q0, tri - window 0 is all-diagonal
    so its exps need the mask tile) at the head of the sync HWDGE FIFO.
  - Engine budget: ACT = exps + Q-chain/V(0..2-era) PSUM copies + den
    extraction; DVE = K/V fill copies, W_O copies, reciprocal, normalize
    muls; GpSimd = partition_broadcast only; fp32 PSUM accumulation
    everywhere; softmax scale 2^-15 inside the exp's free affine.
"""

import numpy as np

try:
    import concourse  # noqa: F401
except ImportError:  # pragma: no cover - harness containers stage it here
    import sys

    sys.path.insert(0, "/opt/trn_rl_repo")

B, S, D, H, DH = 2, 2048, 1024, 16, 64
NCORES = 8
HPC = 4  # heads per core
NPAIR = 2  # head pairs per core
SC = 512  # q-chunk width (scores matmul N)
NQC = S // SC  # 4 q-chunks
NST = S // 128  # 16 s/k/q tiles of 128
NDC = D // 128  # 8 contraction chunks of 128
VO_W = 65  # V columns + ones column
VO_QSTRIDE = 4 * VO_W  # per-head stride inside one quarter's V|ones tile

_cache = {}


def _build_program():
    from contextlib import ExitStack

    import concourse.mybir as mybir
    import concourse.tile as tile
    from concourse import bacc

    f32 = mybir.dt.float32
    f32r = mybir.dt.float32r
    bf16 = mybir.dt.bfloat16
    AF = mybir.ActivationFunctionType

    nc = bacc.Bacc(
        "TRN2", debug=False, target_bir_lowering=False, num_devices=NCORES
    )

    f8 = mybir.dt.float8e4
    xT = nc.dram_tensor("xT", [128, NQC * NDC * SC], bf16, kind="ExternalInput").ap()
    # fp8 DoubleRow operands for the Q/K projections: K=256 per matmul.
    # xq8 cols per quarter/chunk: (512 j) x (2 r) interleaved, r innermost;
    # wqk8 cols per block/chunk: (2 r) x (128 m), m innermost. W scaled by 64
    # (fp8e4m3 normal range); the 1/(64*64) is folded into the exp scale.
    xq8 = nc.dram_tensor(
        "xq8", [128, NQC * 4 * 1024], f8, kind="ExternalInput"
    ).ap()
    wqk8 = nc.dram_tensor(
        "wqk8", [128, 4 * 4 * 256], f8, kind="ExternalInput"
    ).ap()
    wv = nc.dram_tensor("wv", [128, NDC * 256], bf16, kind="ExternalInput").ap()
    wo = nc.dram_tensor("wo", [128, NPAIR * D], bf16, kind="ExternalInput").ap()
    # tri: cols 0:128 = identity, cols 128:256 = strictly-lower tril(-1e30)
    tri = nc.dram_tensor("tri", [128, 256], bf16, kind="ExternalInput").ap()
    out = nc.dram_tensor("out", [S, D], f32, kind="ExternalOutput").ap()

    with tile.TileContext(nc) as tc, ExitStack() as ctx:
        persist = ctx.enter_context(tc.tile_pool(name="persist", bufs=1))
        pt_pool = ctx.enter_context(tc.tile_pool(name="pt", bufs=8))
        den_pool = ctx.enter_context(tc.tile_pool(name="den", bufs=4))
        out_pool = ctx.enter_context(tc.tile_pool(name="outsb", bufs=2))
        ps_pool = ctx.enter_context(tc.tile_pool(name="ps", bufs=2, space="PSUM"))
        pa_pool = ctx.enter_context(tc.tile_pool(name="pa", bufs=2, space="PSUM"))

        # ---- persistent SBUF tensors (per s-quarter where it matters) ----
        x_sb = {
            q: persist.tile([128, NDC * SC], bf16, tag=f"x{q}", name=f"x{q}")
            for q in range(1, NQC)
        }
        x0_sb = [
            persist.tile([128, SC], bf16, tag=f"x0_{dc}", name=f"x0_{dc}")
            for dc in range(NDC)
        ]

        def x_slice(q, dc, lo=0, hi=SC):
            if q == 0:
                return x0_sb[dc][:, lo:hi]
            return x_sb[q][:, dc * SC + lo : dc * SC + hi]

        xq8_sb = persist.tile(
            [128, NQC * 4 * 1024], f8, tag="xq8", name="xq8_sb"
        )
        wqk8_sb = persist.tile([128, 4 * 4 * 256], f8, tag="wqk8", name="wqk8_sb")
        wv_sb = persist.tile([128, NDC * 256], bf16, tag="wv", name="wv_sb")
        wo_sb = persist.tile([128, NPAIR * D], bf16, tag="wo", name="wo_sb")
        trib_sb = persist.tile([128, 256], bf16, tag="trib", name="trib_sb")
        ones_sb = persist.tile([128, 1], f32, tag="ones", name="ones_sb")
        qt_sb = {
            (p, q): persist.tile([128, SC], bf16, tag=f"qt{p}_{q}", name=f"qt{p}_{q}")
            for p in range(NPAIR)
            for q in range(NQC)
        }
        kt_sb = {
            (p, q): persist.tile([128, SC], bf16, tag=f"kt{p}_{q}", name=f"kt{p}_{q}")
            for p in range(NPAIR)
            for q in range(NQC)
        }
        vo_sb = {
            q: persist.tile(
                [128, HPC * VO_QSTRIDE], bf16, tag=f"vo{q}", name=f"vo{q}"
            )
            for q in range(NQC)
        }
        # per-pair denominator rows (par segments side by side), all at
        # partition 0 so partition_broadcast needs no bounce copy
        den1 = [
            persist.tile([1, 2 * SC], f32, tag=f"den1_{p}", name=f"den1_{p}")
            for p in range(NPAIR)
        ]
        denr1 = [
            persist.tile([1, 2 * SC], f32, tag=f"denr1_{p}", name=f"denr1_{p}")
            for p in range(NPAIR)
        ]
        warm_sb = persist.tile([128, SC], bf16, tag="warm", name="warm_sb")
        at_sb = {
            (p, qc): persist.tile(
                [128, SC], bf16, tag=f"at{p}_{qc}", name=f"at{p}_{qc}"
            )
            for p in range(NPAIR)
            for qc in range(NQC)
        }

        # ---- loads: first-matmul dependencies first (sync HWDGE is FIFO);
        # tri right after the first QK operands - window 0 is all-diagonal,
        # so its exps depend on the mask tile ----
        nc.sync.dma_start(wqk8_sb[:], wqk8[:])
        nc.sync.dma_start(xq8_sb[:, 0:4096], xq8[:, 0:4096])
        nc.sync.dma_start(trib_sb[:], tri[:])
        for dc in range(NDC):
            nc.sync.dma_start(x0_sb[dc][:], xT[:, dc * SC : (dc + 1) * SC])
        nc.sync.dma_start(wv_sb[:], wv[:])
        for q in range(1, NQC):
            nc.sync.dma_start(
                xq8_sb[:, q * 4096 : (q + 1) * 4096],
                xq8[:, q * 4096 : (q + 1) * 4096],
            )
        nc.sync.dma_start(x_sb[1][:], xT[:, NDC * SC : 2 * NDC * SC])
        nc.sync.dma_start(wo_sb[:], wo[:])
        for q in range(2, NQC):
            nc.sync.dma_start(x_sb[q][:], xT[:, q * NDC * SC : (q + 1) * NDC * SC])
        nc.vector.memset(warm_sb[:], 0.0)
        nc.vector.memset(ones_sb[:], 1.0)
        for p in range(NPAIR):
            nc.vector.memset(den1[p][:], 1.0)
        for q in range(NQC):
            ones_cols = vo_sb[q].rearrange(
                "p (h s w) -> p h s w", h=HPC, w=VO_W
            )[:, :, :, 64]
            nc.vector.tensor_copy(
                ones_cols, ones_sb[:].to_broadcast((128, HPC, 4))
            )

        # ---- HAM warmup: dummy matmuls during the initial DMA wait flip the
        # PE clock gate to 8/8 before the first real matmul arrives ----
        warm_ps = ps_pool.tile([128, SC], f32, tag="ps", name="warm_ps")
        for i in range(20):
            nc.tensor.matmul(
                warm_ps[:],
                lhsT=warm_sb[:, 0:128],
                rhs=warm_sb[:, 0:SC],
                start=(i == 0),
                stop=(i == 19),
            )
        # force the gpsimd partition_broadcast ucode library to load NOW
        # (during the initial DMA wait) - it is the only gpsimd library the
        # kernel uses, so no mid-kernel ~7us library swaps occur
        warmb_sb = persist.tile([64, 4], f32, tag="warmb", name="warmb_sb")
        nc.gpsimd.partition_broadcast(warmb_sb[:], den1[0][0:1, 0:4])


        # ---- QKV projection thunks ----
        def qk_chain(p, qk, q, on_dve=False):
            dst = qt_sb[(p, q)] if qk == 0 else kt_sb[(p, q)]
            ps = ps_pool.tile([128, SC], f32, tag="ps", name=f"psqk{p}{qk}{q}")
            blk = qk * NPAIR + p
            for c in range(4):
                col = (blk * 4 + c) * 256
                nc.tensor.matmul(
                    ps[:, 0:SC],
                    lhsT=wqk8_sb[:, col : col + 256].rearrange(
                        "p (r m) -> p r m", r=2
                    ),
                    rhs=xq8_sb[
                        :, (q * 4 + c) * 1024 : (q * 4 + c + 1) * 1024
                    ].rearrange("p (r j) -> p r j", r=2),
                    start=(c == 0),
                    stop=(c == 3),
                    perf_mode=mybir.MatmulPerfMode.DoubleRow,
                )
            if on_dve:
                nc.vector.tensor_copy(dst[:], ps[:, 0:SC])
            else:
                nc.scalar.copy(dst[:], ps[:, 0:SC])

        def v_group(q, st4, on_dve=False):
            ps = ps_pool.tile([128, 256], f32, tag="ps", name=f"psv{q}{st4}")
            for dc in range(NDC):
                nc.tensor.matmul(
                    ps[:],
                    lhsT=x_slice(q, dc, st4 * 128, (st4 + 1) * 128),
                    rhs=wv_sb[:, dc * 256 : (dc + 1) * 256],
                    start=(dc == 0),
                    stop=(dc == NDC - 1),
                )
            vo_cols = vo_sb[q].rearrange(
                "p (h s w) -> p h s w", h=HPC, w=VO_W
            )[:, :, st4, 0:64]
            src = ps[:].rearrange("p (h e) -> p h e", e=64)
            if on_dve:
                nc.vector.tensor_copy(vo_cols, src)
            else:
                nc.scalar.copy(vo_cols, src)

        def q_thunks(q):
            return [lambda p=p: qk_chain(p, 0, q) for p in range(NPAIR)]

        def k_thunks(q, on_dve=False):
            return [lambda p=p: qk_chain(p, 1, q, on_dve) for p in range(NPAIR)]

        def v_thunks(q, on_dve=False):
            return [
                lambda st4=st4: v_group(q, st4, on_dve) for st4 in range(4)
            ]

        # ---- W_O projection thunks: one PSUM slot each ----
        outt_tiles = {}

        def wo_half(qc, qt, dc):
            if dc == 0:
                outt_tiles[(qc, qt)] = out_pool.tile(
                    [128, D], f32, tag="outsb", name=f"o{qc}{qt}"
                )
            outt = outt_tiles[(qc, qt)]
            po = ps_pool.tile([128, SC], f32, tag="ps", name=f"po{qc}{qt}{dc}")
            for p in range(NPAIR):
                nc.tensor.matmul(
                    po[:],
                    lhsT=at_sb[(p, qc)][:, qt * 128 : (qt + 1) * 128],
                    rhs=wo_sb[:, p * D + dc * SC : p * D + (dc + 1) * SC],
                    start=(p == 0),
                    stop=(p == NPAIR - 1),
                )
            nc.vector.tensor_copy(outt[:, dc * SC : (dc + 1) * SC], po[:])
            if dc == 1:
                row = (qc * 4 + qt) * 128
                nc.sync.dma_start(out[row : row + 128, :], outt[:])

        def wo_thunks(qc):
            return [
                lambda qt=qt, dc=dc: wo_half(qc, qt, dc)
                for qt in range(4)
                for dc in range(2)
            ]

        # ---- attention window with interleaved fill thunks ----
        def emit_attention(qc, fill=(), epi_qt=None):
            fill = list(fill)
            tail_fill = []
            popped = 0
            nkt = 4 * (qc + 1)
            pa_qc = {
                p: pa_pool.tile([VO_W, 2 * SC], f32, tag="pa", name=f"pa{qc}{p}")
                for p in range(NPAIR)
            }

            def flush(p, kt, ptile):
                j0 = max(0, kt * 128 - qc * SC)
                kq, kst = kt // 4, kt % 4
                for par in range(2):
                    hh = 2 * p + par
                    vbase = hh * VO_QSTRIDE + kst * VO_W
                    nc.tensor.matmul(
                        pa_qc[p][:, par * SC + j0 : (par + 1) * SC],
                        lhsT=vo_sb[kq][:, vbase : vbase + VO_W],
                        rhs=ptile[:, par * SC + j0 : (par + 1) * SC],
                        start=(kt == 0),
                        stop=(kt == nkt - 1),
                    )
                if kt == nkt - 1:
                    # pa(p) complete: extract (ACT) + invert (DVE) this pair's
                    # softmax denominators right away so normalize starts early
                    for par in range(2):
                        seg = slice(par * SC, (par + 1) * SC)
                        nc.scalar.copy(
                            den1[p][0:1, seg],
                            pa_qc[p][64:65, par * SC : (par + 1) * SC],
                        )
                        nc.vector.reciprocal_approx_fast(
                            denr1[p][0:1, seg], den1[p][0:1, seg]
                        )

            pending = []  # (p, kt, ptile) awaiting the P@V matmul
            # spread fills into the drain phase; the final window reserves
            # extra drain-region fills so the PE stays HAM-warm through the
            # normalize chain and into W_O(3)
            nsteps = nkt + (7 if qc == NQC - 1 else 3)
            step = 0

            def pop_fills():
                nonlocal popped
                # front-biased pacing: in-window deadlines (K/V of this very
                # window) sit at the head of the fill list
                want = min(len(fill), -(-len(fill) * (step + 1) // nsteps))
                while popped < want:
                    fill[popped]()
                    popped += 1

            for kt in range(nkt):
                j0 = max(0, kt * 128 - qc * SC)
                kq, kst = kt // 4, kt % 4
                diag = kt * 128 >= qc * SC
                # scores for both pairs first (64-row array tiling: the two
                # par heads run CONCURRENTLY on separate array halves), then
                # the full-array mask matmuls, so the PE switches tiling mode
                # at most twice per kt step
                ps_kt = {}
                for p in range(NPAIR):
                    ps_s = ps_kt[p] = ps_pool.tile(
                        [128, 2 * SC], f32, tag="ps", name=f"pss{qc}{p}{kt}"
                    )
                    for par in range(2):
                        nc.tensor.matmul(
                            ps_s[:, par * SC + j0 : (par + 1) * SC],
                            lhsT=kt_sb[(p, kq)][
                                par * 64 : (par + 1) * 64,
                                kst * 128 : (kst + 1) * 128,
                            ],
                            rhs=qt_sb[(p, qc)][par * 64 : (par + 1) * 64, j0:SC],
                            start=True,
                            stop=not diag,
                            tile_position=(64 * par, 0),
                        )
                for p in range(NPAIR):
                    ps_s = ps_kt[p]
                    if diag:
                        # causal mask on the diagonal 128-block: accumulate
                        # -1e30 strictly-below-diagonal via identity @ mask
                        # (one matmul per par: a PSUM write can't cross banks)
                        for par in range(2):
                            nc.tensor.matmul(
                                ps_s[:, par * SC + j0 : par * SC + j0 + 128],
                                lhsT=trib_sb[:, 0:128],
                                rhs=trib_sb[:, 128:256],
                                start=False,
                                stop=True,
                            )
                    ptile = pt_pool.tile(
                        [128, 2 * SC], bf16, tag="pt", name=f"pt{qc}{p}{kt}"
                    )
                    nc.scalar.activation(
                        ptile.rearrange("p (b n) -> p b n", b=2)[:, :, j0:SC],
                        ps_s.rearrange("p (b n) -> p b n", b=2)[:, :, j0:SC],
                        AF.Exp,
                        scale=0.125 / 4096,
                    )
                    pending.append((p, kt, ptile))
                while len(pending) > 6:
                    flush(*pending.pop(0))
                step = kt + 1
                pop_fills()
            for pend in pending:
                flush(*pend)
                step += 1
                pop_fills()
            # normalize BEFORE the trailing fills so its DVE/GpSimd chain
            # isn't queued behind their PSUM-evacuation copies
            _normalize(qc, pa_qc, epi_qt, tail_fill)
            step = nsteps
            pop_fills()

        def _normalize(qc, pa_qc, epi_qt=None, tail_fill=()):
            tail_fill = list(tail_fill)
            denb = {}
            for p in range(NPAIR):
                denb[p] = den_pool.tile(
                    [64, 2 * SC], f32, tag="denb", name=f"denb{qc}{p}"
                )
                nc.gpsimd.partition_broadcast(denb[p][:], denr1[p][:])
            if epi_qt is None:
                for p in range(NPAIR):
                    for par in range(2):
                        nc.vector.tensor_mul(
                            at_sb[(p, qc)][par * 64 : (par + 1) * 64, :],
                            pa_qc[p][0:64, par * SC : (par + 1) * SC],
                            denb[p][:, par * SC : (par + 1) * SC],
                        )
            else:
                # final window: per-qt muls so each W_O qt-block can start as
                # soon as its 128 q-columns are normalized; reserved fill
                # thunks interleave to keep the PE HAM-warm through the chain
                for qt in range(4):
                    if qt < len(tail_fill):
                        tail_fill[qt]()
                    c0, c1 = qt * 128, (qt + 1) * 128
                    for p in range(NPAIR):
                        for par in range(2):
                            nc.vector.tensor_mul(
                                at_sb[(p, qc)][par * 64 : (par + 1) * 64, c0:c1],
                                pa_qc[p][0:64, par * SC + c0 : par * SC + c1],
                                denb[p][:, par * SC + c0 : par * SC + c1],
                            )
                    epi_qt(qt)

        # ---- main schedule: prologue QK(0) pair-major (scores(p0, kt0) can
        # start after just two chains), then interleaved windows ----
        for p in range(NPAIR):
            qk_chain(p, 0, 0)
            qk_chain(p, 1, 0)
        # fills keyed by DEADLINE: only Q(q+1) must land in window q; K(q) and
        # V(q) pop early inside window q itself (scores kq=q start at kt=4q,
        # flushes trail); W_O(qc') goes anywhere after window qc'. This keeps
        # early windows light and gives the exp-bound late windows PE work.
        fills = {
            0: v_thunks(0, on_dve=True) + q_thunks(1),
            1: k_thunks(1, on_dve=True) + v_thunks(1, on_dve=True)
            + q_thunks(2) + wo_thunks(0),
            2: k_thunks(2, on_dve=True) + v_thunks(2, on_dve=True)
            + q_thunks(3) + wo_thunks(1),
            3: k_thunks(3, on_dve=True) + v_thunks(3, on_dve=True)
            + wo_thunks(2),
        }

        def _wo3_epi(qt):
            wo_half(3, qt, 0)
            wo_half(3, qt, 1)

        for qc in range(NQC):
            emit_attention(
                qc, fill=fills[qc], epi_qt=_wo3_epi if qc == NQC - 1 else None
            )

    nc.compile()
    return nc


def _get_program():
    if "nc" not in _cache:
        _cache["nc"] = _build_program()
    return _cache["nc"]


def _prep_core_inputs(c, residual, W_Q, W_K, W_V, W_O, tri):
    import ml_dtypes

    b = c // 4
    heads = [4 * (c % 4) + i for i in range(HPC)]

    def chunked(w):  # [1024, M] -> [128, NDC*M] chunk-major
        m = w.shape[1]
        return np.ascontiguousarray(
            w.reshape(NDC, 128, m).transpose(1, 0, 2).reshape(128, NDC * m)
        )

    # fp8 DoubleRow wqk blocks: [k, c, r, m] with m innermost, W scaled by 64
    wqk_blocks = []
    for Wt in (W_Q, W_K):
        for p in range(NPAIR):
            h0, h1 = heads[2 * p], heads[2 * p + 1]
            wpair = np.concatenate([Wt[h0].T, Wt[h1].T], axis=1)  # [1024, 128]
            blk = (
                (wpair * 64.0)
                .reshape(4, 2, 128, 128)
                .transpose(2, 0, 1, 3)
                .reshape(128, 4 * 2 * 128)
            )
            wqk_blocks.append(blk)
    wqk8_arr = np.ascontiguousarray(np.concatenate(wqk_blocks, axis=1)).astype(
        ml_dtypes.float8_e4m3
    )

    wv_arr = chunked(np.concatenate([W_V[h].T for h in heads], axis=1))
    wo_arr = np.ascontiguousarray(
        np.concatenate(
            [
                np.concatenate([W_O[heads[2 * p]], W_O[heads[2 * p + 1]]], axis=0)
                for p in range(NPAIR)
            ],
            axis=1,
        )
    )
    xtf = residual[b].T.astype(np.float32)  # [1024, 2048]
    xt = xtf.astype(ml_dtypes.bfloat16)
    xq = np.concatenate(
        [
            np.concatenate(
                [xt[dc * 128 : (dc + 1) * 128, q * SC : (q + 1) * SC]
                 for dc in range(NDC)], axis=1)
            for q in range(NQC)
        ],
        axis=1,
    )
    # fp8 x for DoubleRow QK: [k, q, c, r, j] - K-half r in the middle,
    # j innermost (CoreSim DoubleRow indexes both operands as [p, r, ...])
    xq8_arr = (
        xtf.reshape(4, 2, 128, 4, 512)
        .transpose(2, 3, 0, 1, 4)
        .reshape(128, NQC * 4 * 1024)
        .astype(ml_dtypes.float8_e4m3)
    )
    return {
        "xT": np.ascontiguousarray(xq),
        "xq8": np.ascontiguousarray(xq8_arr),
        "wqk8": wqk8_arr,
        "wv": wv_arr.astype(ml_dtypes.bfloat16),
        "wo": wo_arr.astype(ml_dtypes.bfloat16),
        "tri": tri,
    }


def make_in_maps(residual, W_Q, W_K, W_V, W_O):
    residual = np.asarray(residual, np.float32)
    W_Q, W_K, W_V, W_O = (np.asarray(w, np.float32) for w in (W_Q, W_K, W_V, W_O))
    import ml_dtypes

    # additive causal mask for S^T[k, q] diagonal blocks, applied on the PE:
    # identity (stationary) @ tril(-1e30, -1) accumulated onto the scores
    eye = np.eye(128, dtype=np.float32)
    neg = np.tril(np.full((128, 128), -1e30, np.float32), -1)
    tri = np.concatenate([eye, neg], axis=1).astype(ml_dtypes.bfloat16)
    return [
        _prep_core_inputs(c, residual, W_Q, W_K, W_V, W_O, tri)
        for c in range(NCORES)
    ]


def gather(results):
    out = np.zeros((B, S, D), np.float64)
    for c in range(NCORES):
        out[c // 4] += results[c]["out"].astype(np.float64)
    return out.astype(np.float32)


def kernel(residual, W_Q, W_K, W_V, W_O, **run_kwargs):
    from concourse.bass_utils import run_bass_kernel_spmd

    nc = _get_program()
    in_maps = make_in_maps(residual, W_Q, W_K, W_V, W_O)
    res = run_bass_kernel_spmd(nc, in_maps, core_ids=list(range(NCORES)), **run_kwargs)
    out = gather(res.results)
    if run_kwargs:
        _cache["last_results"] = res
    return out
